# revision 1
# baseline (speedup 1.0000x reference)
"""DFT-D3 dispersion energy kernel for 8 Trainium2 NeuronCores.

Strategy (per sharding hint): shard the 1.6M-edge list across 8 cores
(200k edges each), replicate atoms/tables. Two device launches:

  Launch 1 (CN): edges sorted by i-atom on host into a padded
    [50048, K] slot matrix per core; device computes the D3
    coordination-number counting function per slot, dense-reduces rows
    to per-atom CN partials, AllReduce-psums CN across the 8 cores, and
    computes the per-atom Gaussian C6-interpolation weights W[50048,5].

  Host: gathers W rows to edge endpoints (index marshalling only).

  Launch 2 (energy): plain per-edge arrays; device computes BJ-damped
    pair energies e = c6_ij * u(d) with c6_ij = Wi^T B Wj (B = gathered
    5x5 C6 block), reduces to per-core partials; host sums partials.

All transcendentals use the {Ln, Exp} activation table set only
(sigmoid via exp, sqrt via exp(0.5 ln)) so there is a single ACT table
load in the whole kernel.
"""

import sys

sys.path.insert(0, "/opt/trn_rl_repo")

import numpy as np

import concourse.bacc as bacc
import concourse.bass as bass
import concourse.mybir as mybir
import concourse.tile as tile
from concourse import bass_utils

F32 = mybir.dt.float32
AX = mybir.AluOpType
ACTF = mybir.ActivationFunctionType

# Our only transcendentals are Ln and Exp. Steer the ACT table-load pass
# to the combined natural_log_exp set so the kernel needs exactly one
# table load instead of thrashing between the ln-only and exp-only sets
# (~2.7us per reload).
_orig_get_tables = bacc.get_activation_tables


def _ln_exp_tables(module_arch):
    tables = dict(_orig_get_tables(module_arch))
    out = {}
    for name, funcs in tables.items():
        if name == "natural_log_exp_and_others":
            out[name] = funcs
        else:
            out[name] = funcs - {ACTF.Ln, ACTF.Exp}
    return out


bacc.get_activation_tables = _ln_exp_tables

# D3 constants
K1 = 16.0
K2 = 4.0 / 3.0
K3 = 4.0
A1, A2, S6, S8 = 0.4, 5.0, 1.0, 0.78
CN_CUTOFF2 = 25.0 * 25.0
DISP_CUTOFF2 = 50.0 * 50.0

N_ATOMS = 50000
NP_ATOMS = 50048  # = 128 * 391
GRID_C = 391
N_EDGES = 1_600_000
N_CORES = 8
E_CORE = N_EDGES // N_CORES  # 200000
NREF = 5

# launch-2 chunking: slots per partition per chunk
L2_C = 320
L2_NCH = 5  # 128*320*5 = 204800 >= 200000
E_PAD2 = 128 * L2_C * L2_NCH

_cache = {}


def _runner(nc, out_names):
    """Compile once, return a callable(in_maps) -> list of out dicts."""
    import jax
    from jax.sharding import Mesh, PartitionSpec
    from jax.experimental.shard_map import shard_map
    from concourse import bass2jax

    bass2jax.install_neuronx_cc_hook()

    partition_name = (
        nc.partition_id_tensor.name if nc.partition_id_tensor else None
    )
    in_names = []
    out_avals = []
    zero_outs = []
    onames = []
    for alloc in nc.m.functions[0].allocations:
        if not isinstance(alloc, mybir.MemoryLocationSet):
            continue
        name = alloc.memorylocations[0].name
        if alloc.kind == "ExternalInput":
            if name != partition_name:
                in_names.append(name)
        elif alloc.kind == "ExternalOutput":
            shape = list(alloc.tensor_shape)
            dt = mybir.dt.np(alloc.dtype)
            onames.append(name)
            out_avals.append(jax.core.ShapedArray(shape, dt))
            zero_outs.append(np.zeros(shape, dt))
    n_params = len(in_names)
    all_in = list(in_names) + list(onames)
    if partition_name is not None:
        all_in.append(partition_name)

    from concourse.bass2jax import _bass_exec_p, partition_id_tensor

    def _body(*args):
        operands = list(args)
        if partition_name is not None:
            operands.append(partition_id_tensor())
        outs = _bass_exec_p.bind(
            *operands,
            out_avals=tuple(out_avals),
            in_names=tuple(all_in),
            out_names=tuple(onames),
            lowering_input_output_aliases=(),
            sim_require_finite=True,
            sim_require_nnan=True,
            nc=nc,
        )
        return tuple(outs)

    devices = jax.devices()[:N_CORES]
    mesh = Mesh(np.asarray(devices), ("core",))
    donate = tuple(range(n_params, n_params + len(onames)))
    sharded = jax.jit(
        shard_map(
            _body,
            mesh=mesh,
            in_specs=(PartitionSpec("core"),) * (n_params + len(onames)),
            out_specs=(PartitionSpec("core"),) * len(onames),
            check_rep=False,
        ),
        donate_argnums=donate,
        keep_unused=True,
    )

    def _concat(in_maps):
        per_core = [[np.asarray(m[n]) for n in in_names] for m in in_maps]
        return [
            np.concatenate([per_core[c][i] for c in range(N_CORES)], axis=0)
            for i in range(n_params)
        ]

    def _zeros():
        return [
            np.zeros((N_CORES * z.shape[0], *z.shape[1:]), z.dtype)
            for z in zero_outs
        ]

    def _unpack(out_arrs):
        return [
            {
                n: np.asarray(out_arrs[i]).reshape(
                    N_CORES, *out_avals[i].shape
                )[c]
                for i, n in enumerate(onames)
            }
            for c in range(N_CORES)
        ]

    def run(in_maps):
        return _unpack(sharded(*_concat(in_maps), *_zeros()))

    def run_timed(in_maps, iters=3):
        """Pre-stage inputs on device, time execute-only. Returns
        (results, best_seconds)."""
        import time
        from jax.sharding import NamedSharding

        sh = NamedSharding(mesh, PartitionSpec("core"))
        staged = [jax.device_put(a, sh) for a in _concat(in_maps)]
        out = sharded(*staged, *_zeros())  # warm
        jax.block_until_ready(out)
        best = float("inf")
        for _ in range(iters):
            z = [jax.device_put(a, sh) for a in _zeros()]
            jax.block_until_ready(z)
            t0 = time.perf_counter()
            out = sharded(*staged, *z)
            jax.block_until_ready(out)
            best = min(best, time.perf_counter() - t0)
        return _unpack(out), best

    run.run_timed = run_timed
    return run


# ---------------------------------------------------------------- launch 1
def _register_consts(nc, values):
    for value in values:
        t = nc.alloc_sbuf_tensor(f"constx-f32-{value}", [128, 1], F32)
        nc.gpsimd.memset(t.ap(), value)
        nc.const_aps.aps[(F32, value)] = t.ap()
    nc.all_engine_barrier()


def build_launch1(K):
    """CN pass: padded slot matrix -> cn grid -> AllReduce -> W.

    k-major layout: pjt[k, atom, 4] (j-side per slot), slf[atom, 4]
    (i-side, constant per atom, read via broadcast APs). Compute runs
    full-width [128, Kc*391] per chunk to amortize DVE instruction
    overhead.
    """
    nc = bacc.Bacc(None, target_bir_lowering=False, num_devices=N_CORES)
    _register_consts(nc, [1e-20, K1])
    pjt = nc.dram_tensor("pjt", [K, NP_ATOMS, 4], F32, kind="ExternalInput")
    slf = nc.dram_tensor("slf", [NP_ATOMS, 4], F32, kind="ExternalInput")
    cnr = nc.dram_tensor("cnr", [NP_ATOMS, NREF], F32, kind="ExternalInput")
    wout = nc.dram_tensor("wout", [NP_ATOMS, NREF], F32, kind="ExternalOutput")
    cnout = nc.dram_tensor("cnout", [128, GRID_C], F32, kind="ExternalOutput")

    KC = 4  # k-slots per chunk
    G = GRID_C

    with tile.TileContext(nc) as tc:
        with (
            tc.tile_pool(name="io", bufs=2) as io,
            tc.tile_pool(name="tmp", bufs=1) as tp,
            tc.tile_pool(name="acc", bufs=1) as ac,
            tc.tile_pool(name="dram", bufs=1, space="DRAM") as dr,
        ):
            sl = ac.tile([128, G * 4], F32)
            nc.sync.dma_start(
                sl[:], slf[:].rearrange("(p c) f -> p (c f)", p=128)
            )
            slv = sl[:].rearrange("p (c f) -> p c f", f=4)

            def selfb(f, kc):
                # [128, G] field -> [128, kc, G] broadcast over k
                return (
                    slv[:, :, f]
                    .to_broadcast([128, G, kc])
                    .rearrange("p c k -> p k c")
                )

            cng = ac.tile([128, GRID_C], F32)
            nc.vector.memset(cng[:], 0.0)
            k0 = 0
            while k0 < K:
                kc = min(KC, K - k0)
                t = io.tile([128, KC * G * 4], F32, tag="pjin")
                for ki in range(kc):
                    nc.sync.dma_start(
                        t[:].rearrange("p (k m) -> p k m", k=KC)[:, ki, :],
                        pjt[k0 + ki].rearrange("(p c) f -> p (c f)", p=128),
                    )
                v = t[:].rearrange("p (k c f) -> p k c f", k=KC, f=4)[:, :kc]
                S = kc * G
                dx = tp.tile([128, KC * G], F32, tag="dx")
                dy = tp.tile([128, KC * G], F32, tag="dy")
                d2 = tp.tile([128, KC * G], F32, tag="d2")
                rr = tp.tile([128, KC * G], F32, tag="rr")
                dxv = dx[:, :S].rearrange("p (k c) -> p k c", k=kc)
                dyv = dy[:, :S].rearrange("p (k c) -> p k c", k=kc)
                d2v = d2[:, :S].rearrange("p (k c) -> p k c", k=kc)
                rrv = rr[:, :S].rearrange("p (k c) -> p k c", k=kc)
                nc.vector.tensor_tensor(dxv, v[:, :, :, 0], selfb(0, kc), op=AX.subtract)
                nc.vector.tensor_tensor(dyv, v[:, :, :, 1], selfb(1, kc), op=AX.subtract)
                nc.vector.tensor_tensor(rrv, v[:, :, :, 3], selfb(3, kc), op=AX.add)
                nc.vector.tensor_tensor(d2[:, :S], dx[:, :S], dx[:, :S], op=AX.mult)
                nc.vector.tensor_tensor(dx[:, :S], dy[:, :S], dy[:, :S], op=AX.mult)
                nc.vector.tensor_tensor(d2[:, :S], d2[:, :S], dx[:, :S], op=AX.add)
                nc.vector.tensor_tensor(dyv, v[:, :, :, 2], selfb(2, kc), op=AX.subtract)
                nc.vector.tensor_tensor(dx[:, :S], dy[:, :S], dy[:, :S], op=AX.mult)
                nc.vector.tensor_tensor(d2[:, :S], d2[:, :S], dx[:, :S], op=AX.add)
                ln_d2 = tp.tile([128, KC * G], F32, tag="lnd2")
                ln_rr = tp.tile([128, KC * G], F32, tag="lnrr")
                nc.scalar.activation(ln_d2[:, :S], d2[:, :S], ACTF.Ln, bias=1e-20)
                nc.scalar.activation(ln_rr[:, :S], rr[:, :S], ACTF.Ln)
                arg = tp.tile([128, KC * G], F32, tag="arg")
                nc.vector.tensor_scalar(arg[:, :S], ln_d2[:, :S], -0.5, None, op0=AX.mult)
                nc.vector.tensor_tensor(arg[:, :S], arg[:, :S], ln_rr[:, :S], op=AX.add)
                tt = tp.tile([128, KC * G], F32, tag="tt")
                nc.scalar.activation(tt[:, :S], arg[:, :S], ACTF.Exp)
                g = tp.tile([128, KC * G], F32, tag="g")
                nc.scalar.activation(g[:, :S], tt[:, :S], ACTF.Exp, bias=K1, scale=-K1 * K2)
                nc.vector.tensor_scalar(g[:, :S], g[:, :S], 1.0, None, op0=AX.add)
                rec = tp.tile([128, KC * G], F32, tag="rec")
                nc.vector.reciprocal(rec[:, :S], g[:, :S])
                msk = tp.tile([128, KC * G], F32, tag="msk")
                nc.vector.tensor_scalar(msk[:, :S], d2[:, :S], CN_CUTOFF2, None, op0=AX.is_lt)
                nc.vector.tensor_tensor(rec[:, :S], rec[:, :S], msk[:, :S], op=AX.mult)
                # reduce over k (strided innermost) and accumulate
                part = tp.tile([128, G], F32, tag="part")
                nc.vector.tensor_reduce(
                    part[:],
                    rec[:, :S]
                    .rearrange("p (k c) -> p k c", k=kc)
                    .rearrange("p k c -> p c k"),
                    axis=mybir.AxisListType.X,
                    op=AX.add,
                )
                nc.vector.tensor_tensor(cng[:], cng[:], part[:], op=AX.add)
                k0 += kc

            # AllReduce cn across cores (psum)
            cin = dr.tile([128, GRID_C], F32)
            cout = dr.tile([128, GRID_C], F32)
            nc.sync.dma_start(cin[:], cng[:])
            nc.gpsimd.collective_compute(
                "AllReduce",
                AX.add,
                replica_groups=[list(range(N_CORES))],
                ins=[cin[:].opt()],
                outs=[cout[:].opt()],
            )
            cn = ac.tile([128, GRID_C], F32)
            nc.sync.dma_start(cn[:], cout[:])
            nc.sync.dma_start(cnout[:], cn[:])

            # ---- W build (per atom) ----
            G = GRID_C
            cr = ac.tile([128, G * NREF], F32)
            nc.sync.dma_start(
                cr[:], cnr[:].rearrange("(p c) r -> p (c r)", p=128)
            )
            crv = cr[:].rearrange("p (c r) -> p c r", r=NREF)
            gw = ac.tile([128, G * NREF], F32)
            gwv = gw[:].rearrange("p (c r) -> p c r", r=NREF)
            mk = ac.tile([128, G * NREF], F32)
            mkv = mk[:].rearrange("p (c r) -> p c r", r=NREF)
            dr_ = tp.tile([128, G], F32, tag="wdr")
            for r in range(NREF):
                nc.vector.tensor_tensor(dr_[:], cn[:], crv[:, :, r], op=AX.subtract)
                nc.vector.tensor_tensor(dr_[:], dr_[:], dr_[:], op=AX.mult)
                nc.scalar.activation(gwv[:, :, r], dr_[:], ACTF.Exp, scale=-K3)
            nc.vector.tensor_scalar(mk[:], cr[:], 0.0, None, op0=AX.is_ge)
            nc.vector.tensor_tensor(gw[:], gw[:], mk[:], op=AX.mult)
            norm = tp.tile([128, G], F32, tag="wnorm")
            nc.vector.tensor_reduce(
                norm[:], gwv[:, :, :], axis=mybir.AxisListType.X, op=AX.add
            )
            # maxv = ref4 if ref4>=0 else ref3
            maxv = tp.tile([128, G], F32, tag="wmaxv")
            t1 = tp.tile([128, G], F32, tag="wt1")
            nc.vector.tensor_tensor(
                maxv[:], crv[:, :, NREF - 1], mkv[:, :, NREF - 1], op=AX.mult
            )
            nc.vector.tensor_scalar(
                t1[:], mkv[:, :, NREF - 1], -1.0, 1.0, op0=AX.mult, op1=AX.add
            )
            nc.vector.tensor_tensor(t1[:], t1[:], crv[:, :, NREF - 2], op=AX.mult)
            nc.vector.tensor_tensor(maxv[:], maxv[:], t1[:], op=AX.add)
            # usefb / denom
            usefb = tp.tile([128, G], F32, tag="wufb")
            nc.vector.tensor_scalar(usefb[:], norm[:], 1e-30, None, op0=AX.is_le)
            nofb = tp.tile([128, G], F32, tag="wnfb")
            nc.vector.tensor_scalar(
                nofb[:], usefb[:], -1.0, 1.0, op0=AX.mult, op1=AX.add
            )
            nc.vector.tensor_scalar(norm[:], norm[:], 1e-30, None, op0=AX.max)
            rn = tp.tile([128, G], F32, tag="wrn")
            nc.vector.reciprocal(rn[:], norm[:])
            nc.vector.tensor_tensor(rn[:], rn[:], nofb[:], op=AX.mult)
            wpack = ac.tile([128, G * NREF], F32)
            wv = wpack[:].rearrange("p (c r) -> p c r", r=NREF)
            fb = tp.tile([128, G], F32, tag="wfb")
            for r in range(NREF):
                nc.vector.tensor_tensor(fb[:], crv[:, :, r], maxv[:], op=AX.is_equal)
                nc.vector.tensor_tensor(fb[:], fb[:], mkv[:, :, r], op=AX.mult)
                nc.vector.tensor_tensor(fb[:], fb[:], usefb[:], op=AX.mult)
                nc.vector.tensor_tensor(
                    wv[:, :, r], gwv[:, :, r], rn[:], op=AX.mult
                )
                nc.vector.tensor_tensor(
                    wv[:, :, r], wv[:, :, r], fb[:], op=AX.add
                )
            nc.sync.dma_start(
                wout[:].rearrange("(p c) r -> p (c r)", p=128), wpack[:]
            )
    nc.finalize()
    return nc


# ---------------------------------------------------------------- launch 2
def build_launch2():
    nc = bacc.Bacc(None, target_bir_lowering=False, num_devices=N_CORES)
    # geo: xi yi zi xj yj zj r4i r4j
    geo = nc.dram_tensor("geo", [E_PAD2, 8], F32, kind="ExternalInput")
    wij = nc.dram_tensor("wij", [E_PAD2, 2 * NREF], F32, kind="ExternalInput")
    c6b = nc.dram_tensor("c6b", [E_PAD2, 25], F32, kind="ExternalInput")
    eout = nc.dram_tensor("eout", [128, 1], F32, kind="ExternalOutput")

    C = L2_C
    with tile.TileContext(nc) as tc:
        with (
            tc.tile_pool(name="io", bufs=2) as io,
            tc.tile_pool(name="tmp", bufs=1) as tp,
            tc.tile_pool(name="acc", bufs=1) as ac,
        ):
            eacc = ac.tile([128, 1], F32)
            nc.vector.memset(eacc[:], 0.0)
            for ch in range(L2_NCH):
                e0 = ch * 128 * C
                g = io.tile([128, C * 8], F32, tag="geo")
                nc.sync.dma_start(
                    g[:],
                    geo[e0 : e0 + 128 * C, :].rearrange(
                        "(p c) f -> p (c f)", p=128
                    ),
                )
                gv = g[:].rearrange("p (c f) -> p c f", f=8)
                w = io.tile([128, C * 2 * NREF], F32, tag="wij")
                nc.sync.dma_start(
                    w[:],
                    wij[e0 : e0 + 128 * C, :].rearrange(
                        "(p c) f -> p (c f)", p=128
                    ),
                )
                wvv = w[:].rearrange("p (c f) -> p c f", f=2 * NREF)
                cb = io.tile([128, C * 25], F32, tag="c6b")
                nc.sync.dma_start(
                    cb[:],
                    c6b[e0 : e0 + 128 * C, :].rearrange(
                        "(p c) f -> p (c f)", p=128
                    ),
                )
                # d2
                dx = tp.tile([128, C], F32, tag="dx")
                dy = tp.tile([128, C], F32, tag="dy")
                d2 = tp.tile([128, C], F32, tag="d2")
                nc.vector.tensor_tensor(dx[:], gv[:, :, 0], gv[:, :, 3], op=AX.subtract)
                nc.vector.tensor_tensor(dy[:], gv[:, :, 1], gv[:, :, 4], op=AX.subtract)
                nc.vector.tensor_tensor(d2[:], dx[:], dx[:], op=AX.mult)
                nc.vector.tensor_tensor(dx[:], dy[:], dy[:], op=AX.mult)
                nc.vector.tensor_tensor(d2[:], d2[:], dx[:], op=AX.add)
                nc.vector.tensor_tensor(dy[:], gv[:, :, 2], gv[:, :, 5], op=AX.subtract)
                nc.vector.tensor_tensor(dx[:], dy[:], dy[:], op=AX.mult)
                nc.vector.tensor_tensor(d2[:], d2[:], dx[:], op=AX.add)
                nc.vector.tensor_scalar(d2[:], d2[:], 1e-20, None, op0=AX.add)
                # q = r4i*r4j ; sq = sqrt(q) = exp(0.5 ln q); f = A1*sqrt(3q)+A2
                q = tp.tile([128, C], F32, tag="q")
                nc.vector.tensor_tensor(q[:], gv[:, :, 6], gv[:, :, 7], op=AX.mult)
                lnq = tp.tile([128, C], F32, tag="lnq")
                nc.scalar.activation(lnq[:], q[:], ACTF.Ln)
                sq = tp.tile([128, C], F32, tag="sq")
                nc.scalar.activation(sq[:], lnq[:], ACTF.Exp, scale=0.5)
                f = tp.tile([128, C], F32, tag="f")
                nc.vector.tensor_scalar(
                    f[:], sq[:], A1 * np.sqrt(3.0), A2, op0=AX.mult, op1=AX.add
                )
                f2 = tp.tile([128, C], F32, tag="f2")
                nc.vector.tensor_tensor(f2[:], f[:], f[:], op=AX.mult)
                f4 = tp.tile([128, C], F32, tag="f4")
                nc.vector.tensor_tensor(f4[:], f2[:], f2[:], op=AX.mult)
                f6 = tp.tile([128, C], F32, tag="f6")
                nc.vector.tensor_tensor(f6[:], f4[:], f2[:], op=AX.mult)
                nc.vector.tensor_tensor(f4[:], f4[:], f4[:], op=AX.mult)  # f8
                d4 = tp.tile([128, C], F32, tag="d4")
                nc.vector.tensor_tensor(d4[:], d2[:], d2[:], op=AX.mult)
                d6 = tp.tile([128, C], F32, tag="d6")
                nc.vector.tensor_tensor(d6[:], d4[:], d2[:], op=AX.mult)
                nc.vector.tensor_tensor(d4[:], d4[:], d4[:], op=AX.mult)  # d8
                nc.vector.tensor_tensor(d6[:], d6[:], f6[:], op=AX.add)
                nc.vector.tensor_tensor(d4[:], d4[:], f4[:], op=AX.add)
                r6 = tp.tile([128, C], F32, tag="r6")
                nc.vector.reciprocal(r6[:], d6[:])
                r8 = tp.tile([128, C], F32, tag="r8")
                nc.vector.reciprocal(r8[:], d4[:])
                # u = (S6*r6 + 3*S8*q*r8) * (d2<2500)   [sign applied at end]
                nc.vector.tensor_tensor(r8[:], r8[:], q[:], op=AX.mult)
                nc.vector.tensor_scalar(r8[:], r8[:], 3.0 * S8, None, op0=AX.mult)
                nc.vector.tensor_scalar(r6[:], r6[:], S6, None, op0=AX.mult)
                nc.vector.tensor_tensor(r6[:], r6[:], r8[:], op=AX.add)
                m50 = tp.tile([128, C], F32, tag="m50")
                nc.vector.tensor_scalar(
                    m50[:], d2[:], DISP_CUTOFF2, None, op0=AX.is_lt
                )
                nc.vector.tensor_tensor(r6[:], r6[:], m50[:], op=AX.mult)
                # einsum: c6 = sum_ab Wi_a Wj_b B_ab
                op = tp.tile([128, C * 25], F32, tag="op")
                opv = op[:].rearrange("p (c a b) -> p c a b", a=NREF, b=NREF)
                wiB = wvv[:, :, 0:NREF].to_broadcast([128, C, NREF, NREF])
                wjB = (
                    wvv[:, :, NREF : 2 * NREF]
                    .to_broadcast([128, C, NREF, NREF])
                    .rearrange("p c b a -> p c a b")
                )
                nc.vector.tensor_tensor(opv, wiB, wjB, op=AX.mult)
                nc.vector.tensor_tensor(op[:], op[:], cb[:], op=AX.mult)
                c6 = tp.tile([128, C], F32, tag="c6")
                nc.vector.tensor_reduce(
                    c6[:],
                    op[:].rearrange("p (c e) -> p c e", e=25),
                    axis=mybir.AxisListType.X,
                    op=AX.add,
                )
                nc.vector.tensor_tensor(c6[:], c6[:], r6[:], op=AX.mult)
                er = tp.tile([128, 1], F32, tag="er")
                nc.vector.tensor_reduce(
                    er[:], c6[:], axis=mybir.AxisListType.X, op=AX.add
                )
                nc.vector.tensor_tensor(eacc[:], eacc[:], er[:], op=AX.add)
            nc.vector.tensor_scalar(eacc[:], eacc[:], -0.5, None, op0=AX.mult)
            nc.sync.dma_start(eout[:], eacc[:])
    nc.finalize()
    return nc


# ---------------------------------------------------------------- host side
def _prep(positions, numbers, edges_i, edges_j, rcov, r4r2, c6_table, cn_ref):
    """Host-side sharding + index marshalling. Returns (K, l1_maps, meta)."""
    pos = np.zeros((NP_ATOMS, 3), np.float32)
    pos[:N_ATOMS] = positions
    # pad atoms far away so any accidental reference is masked out
    pos[N_ATOMS:] = 1.0e4
    num = np.zeros(NP_ATOMS, np.int32)
    num[:N_ATOMS] = numbers
    rcov_a = rcov[num].astype(np.float32)
    r4r2_a = r4r2[num].astype(np.float32)
    cnr_a = cn_ref[num].astype(np.float32)  # [NP, 5]

    cores = []
    Kmax = 1
    for c in range(N_CORES):
        ei = edges_i[c * E_CORE : (c + 1) * E_CORE].astype(np.int64)
        ej = edges_j[c * E_CORE : (c + 1) * E_CORE].astype(np.int64)
        order = np.argsort(ei, kind="stable")
        ei, ej = ei[order], ej[order]
        counts = np.bincount(ei, minlength=NP_ATOMS)
        Kmax = max(Kmax, int(counts.max()))
        cores.append((ei, ej, counts))
    K = int(Kmax)

    l1_maps = []
    metas = []
    for c in range(N_CORES):
        ei, ej, counts = cores[c]
        starts = np.zeros(NP_ATOMS, np.int64)
        starts[1:] = np.cumsum(counts)[:-1]
        kpos = np.arange(E_CORE, dtype=np.int64) - starts[ei]
        # k-major j-side slots [K, NP, 4]; pad xj=1e3 (masked), rcov=0.5
        pjt = np.zeros((K, NP_ATOMS, 4), np.float32)
        pjt[:, :, 0] = 1.0e3
        pjt[:, :, 3] = 0.5
        pjt[kpos, ei, 0:3] = pos[ej]
        pjt[kpos, ei, 3] = rcov_a[ej]
        slfa = np.zeros((NP_ATOMS, 4), np.float32)
        slfa[:, 0:3] = pos
        slfa[:, 3] = rcov_a
        l1_maps.append(dict(pjt=pjt, slf=slfa, cnr=cnr_a))
        metas.append((ei, ej))
    return K, l1_maps, metas


def kernel(positions, numbers, edges_i, edges_j, rcov, r4r2, c6_table,
           cn_ref, _times=None):
    K, l1_maps, metas = _prep(
        positions, numbers, edges_i, edges_j, rcov, r4r2, c6_table, cn_ref
    )

    if ("l1", K) not in _cache:
        _cache[("l1", K)] = _runner(build_launch1(K), ["wout", "cnout"])
    run1 = _cache[("l1", K)]
    if _times is not None:
        res1, t1 = run1.run_timed(l1_maps)
        _times.append(t1)
    else:
        res1 = run1(l1_maps)
    W = res1[0]["wout"]  # [NP_ATOMS, 5] (identical on all cores)

    num = np.zeros(NP_ATOMS, np.int32)
    num[:N_ATOMS] = numbers
    pos = np.zeros((NP_ATOMS, 3), np.float32)
    pos[:N_ATOMS] = positions
    r4r2_a = r4r2[num].astype(np.float32)
    c6f = np.ascontiguousarray(c6_table.reshape(95 * 95, 25).astype(np.float32))

    l2_maps = []
    for c in range(N_CORES):
        ei, ej = metas[c]
        geo = np.zeros((E_PAD2, 8), np.float32)
        geo[:, 3] = 1.0e3  # pad: far apart -> masked
        geo[:, 6:8] = 1.0  # pad: ln(1)=0 safe
        geo[:E_CORE, 0:3] = pos[ei]
        geo[:E_CORE, 3:6] = pos[ej]
        geo[:E_CORE, 6] = r4r2_a[ei]
        geo[:E_CORE, 7] = r4r2_a[ej]
        wij = np.zeros((E_PAD2, 10), np.float32)
        wij[:E_CORE, 0:5] = W[ei]
        wij[:E_CORE, 5:10] = W[ej]
        c6b = np.zeros((E_PAD2, 25), np.float32)
        pair = num[ei].astype(np.int64) * 95 + num[ej]
        c6b[:E_CORE] = c6f[pair]
        l2_maps.append(dict(geo=geo, wij=wij, c6b=c6b))

    if "l2" not in _cache:
        _cache["l2"] = _runner(build_launch2(), ["eout"])
    run2 = _cache["l2"]
    if _times is not None:
        res2, t2 = run2.run_timed(l2_maps)
        _times.append(t2)
    else:
        res2 = run2(l2_maps)
    total = sum(float(res2[c]["eout"].sum()) for c in range(N_CORES))
    return np.float32(total)



# revision 3
# speedup vs baseline: 1.8843x; 1.8843x over previous
"""DFT-D3 dispersion energy kernel for 8 Trainium2 NeuronCores.

Strategy: partition EDGES BY OWNER ATOM BLOCK (core c owns atoms
[c*6250, (c+1)*6250) and every edge whose i-endpoint lands there, ~200k
edges/core).  Coordination numbers for owned atoms then complete
locally -> NO AllReduce at all.  Two device launches:

  Launch 1 (CN+W): per-core atoms are sorted by local degree
    (descending) and laid out rank-major on a [128 x 49] grid; edge
    j-side data goes into a degree-truncated slot structure (level
    chunks of KC=8 with per-chunk column widths), ~1.13x padding
    instead of 4x for a rectangular grid.  All fields are PLANAR
    (separate x/y/z/rcov planes) so every DVE op is unit-stride.
    Squares/exp/ln run on the scalar (ACT) engine using the single
    natural_log_exp table (sigmoid via exp, sqrt via exp(0.5 ln)).
    Device computes per-atom CN, then the Gaussian C6-interpolation
    weights W[6272, 5].  Host gathers W rows to edge endpoints.

  Launch 2 (energy): flat per-edge planar bf16 streams (positions,
    r4r2, Wi/Wj, 5x5 C6 blocks).  Geometry/damping chain in fp32 with
    squares+sqrt on ACT (sqrt_and_others table), the 25-wide einsum in
    bf16 split across DVE (outer-product, reduce) and the Pool engine
    (product with the C6 block), and a fused tensor_tensor_reduce for
    the per-chunk energy accumulation.  Cutoff masks are dropped: the
    sigmoid/damping tails beyond the cutoffs contribute ~1e-6 relative.

Host work is index marshalling only (sorts, gathers, layout packing);
all arithmetic of the reference runs on device.
"""

import sys

sys.path.insert(0, "/opt/trn_rl_repo")

import numpy as np
import ml_dtypes

BF16NP = ml_dtypes.bfloat16

import concourse.bacc as bacc
import concourse.bass as bass
import concourse.mybir as mybir
import concourse.tile as tile
from concourse import bass_utils

F32 = mybir.dt.float32
BF16 = mybir.dt.bfloat16
AX = mybir.AluOpType
ACTF = mybir.ActivationFunctionType

# Steer the ACT table-load pass: launch 1 only ever needs {Ln, Exp,
# Square} (one natural_log_exp table), launch 2 only {Sqrt, Square}
# (one sqrt table).  Strip Ln/Exp from every other set so the chooser
# can't thrash.
_orig_get_tables = bacc.get_activation_tables


def _ln_exp_tables(module_arch):
    tables = dict(_orig_get_tables(module_arch))
    out = {}
    for name, funcs in tables.items():
        if name == "natural_log_exp_and_others":
            out[name] = funcs
        else:
            out[name] = funcs - {ACTF.Ln, ACTF.Exp}
    return out


bacc.get_activation_tables = _ln_exp_tables

# D3 constants
K1 = 16.0
K2 = 4.0 / 3.0
K3 = 4.0
A1, A2, S6, S8 = 0.4, 5.0, 1.0, 0.78

N_ATOMS = 50000
N_CORES = 8
ABLK = 6250          # atoms owned per core
A_PAD = 6272         # = 128 * 49
G = 49               # atom-grid columns
KC = 8               # slot levels per chunk
N_EDGES = 1_600_000
NREF = 5

# launch-2 chunking
L2_C = 400
L2_NCH = 4
E_PAD2 = 128 * L2_C * L2_NCH  # 204800

_cache = {}


def _runner(nc, out_names):
    """Compile once, return a callable(in_maps) -> list of out dicts."""
    import jax
    from jax.sharding import Mesh, PartitionSpec
    from jax.experimental.shard_map import shard_map
    from concourse import bass2jax

    bass2jax.install_neuronx_cc_hook()

    partition_name = (
        nc.partition_id_tensor.name if nc.partition_id_tensor else None
    )
    in_names = []
    out_avals = []
    zero_outs = []
    onames = []
    for alloc in nc.m.functions[0].allocations:
        if not isinstance(alloc, mybir.MemoryLocationSet):
            continue
        name = alloc.memorylocations[0].name
        if alloc.kind == "ExternalInput":
            if name != partition_name:
                in_names.append(name)
        elif alloc.kind == "ExternalOutput":
            shape = list(alloc.tensor_shape)
            dt = mybir.dt.np(alloc.dtype)
            onames.append(name)
            out_avals.append(jax.core.ShapedArray(shape, dt))
            zero_outs.append(np.zeros(shape, dt))
    n_params = len(in_names)
    all_in = list(in_names) + list(onames)
    if partition_name is not None:
        all_in.append(partition_name)

    from concourse.bass2jax import _bass_exec_p, partition_id_tensor

    def _body(*args):
        operands = list(args)
        if partition_name is not None:
            operands.append(partition_id_tensor())
        outs = _bass_exec_p.bind(
            *operands,
            out_avals=tuple(out_avals),
            in_names=tuple(all_in),
            out_names=tuple(onames),
            lowering_input_output_aliases=(),
            sim_require_finite=True,
            sim_require_nnan=True,
            nc=nc,
        )
        return tuple(outs)

    devices = jax.devices()[:N_CORES]
    mesh = Mesh(np.asarray(devices), ("core",))
    donate = tuple(range(n_params, n_params + len(onames)))
    sharded = jax.jit(
        shard_map(
            _body,
            mesh=mesh,
            in_specs=(PartitionSpec("core"),) * (n_params + len(onames)),
            out_specs=(PartitionSpec("core"),) * len(onames),
            check_rep=False,
        ),
        donate_argnums=donate,
        keep_unused=True,
    )

    def _concat(in_maps):
        per_core = [[np.asarray(m[n]) for n in in_names] for m in in_maps]
        return [
            np.concatenate([per_core[c][i] for c in range(N_CORES)], axis=0)
            for i in range(n_params)
        ]

    def _zeros():
        return [
            np.zeros((N_CORES * z.shape[0], *z.shape[1:]), z.dtype)
            for z in zero_outs
        ]

    def _unpack(out_arrs):
        return [
            {
                n: np.asarray(out_arrs[i]).reshape(
                    N_CORES, *out_avals[i].shape
                )[c]
                for i, n in enumerate(onames)
            }
            for c in range(N_CORES)
        ]

    def run(in_maps):
        return _unpack(sharded(*_concat(in_maps), *_zeros()))

    def run_timed(in_maps, iters=3):
        """Pre-stage inputs on device, time execute-only. Returns
        (results, best_seconds)."""
        import time
        from jax.sharding import NamedSharding

        sh = NamedSharding(mesh, PartitionSpec("core"))
        staged = [jax.device_put(a, sh) for a in _concat(in_maps)]
        out = sharded(*staged, *_zeros())  # warm
        jax.block_until_ready(out)
        best = float("inf")
        for _ in range(iters):
            z = [jax.device_put(a, sh) for a in _zeros()]
            jax.block_until_ready(z)
            t0 = time.perf_counter()
            out = sharded(*staged, *z)
            jax.block_until_ready(out)
            best = min(best, time.perf_counter() - t0)
        return _unpack(out), best

    run.run_timed = run_timed
    return run


def _register_consts(nc, values):
    for value in values:
        t = nc.alloc_sbuf_tensor(f"constx-f32-{value}", [128, 1], F32)
        nc.gpsimd.memset(t.ap(), value)
        nc.const_aps.aps[(F32, value)] = t.ap()
    nc.all_engine_barrier()


# ---------------------------------------------------------------- launch 1
def build_launch1(widths):
    """CN pass on the degree-truncated slot grid, then W build.

    widths: per level-chunk column counts (same on all cores).  DRAM
    planes pjx/pjy/pjz/pjr are [128, TOT] with chunk t occupying
    columns [off_t, off_t + KC*m_t), k-major within the chunk.
    """
    nc = bacc.Bacc(None, target_bir_lowering=False, num_devices=N_CORES)
    _register_consts(nc, [1e-20, K1])
    TOT = sum(KC * m for m in widths)
    pjx = nc.dram_tensor("pjx", [128, TOT], F32, kind="ExternalInput")
    pjy = nc.dram_tensor("pjy", [128, TOT], F32, kind="ExternalInput")
    pjz = nc.dram_tensor("pjz", [128, TOT], F32, kind="ExternalInput")
    pjr = nc.dram_tensor("pjr", [128, TOT], F32, kind="ExternalInput")
    slf = nc.dram_tensor("slf", [128, 4 * G], F32, kind="ExternalInput")
    cnrt = nc.dram_tensor("cnrt", [128, NREF * G], F32, kind="ExternalInput")
    wout = nc.dram_tensor("wout", [128, NREF * G], F32, kind="ExternalOutput")

    SMAX = KC * widths[0]

    with tile.TileContext(nc) as tc:
        with (
            tc.tile_pool(name="io", bufs=2) as io,
            tc.tile_pool(name="tmp", bufs=1) as tp,
            tc.tile_pool(name="acc", bufs=1) as ac,
        ):
            sl = ac.tile([128, 4 * G], F32)
            nc.sync.dma_start(sl[:], slf[:])
            cn = ac.tile([128, G], F32)
            nc.vector.memset(cn[:], 0.0)

            def selfb(f, m):
                # [128, m] self plane -> [128, KC, m] broadcast over k
                return (
                    sl[:, f * G : f * G + m]
                    .to_broadcast([128, m, KC])
                    .rearrange("p c k -> p k c")
                )

            off = 0
            for m in widths:
                S = KC * m
                xj = io.tile([128, SMAX], F32, tag="xj")
                yj = io.tile([128, SMAX], F32, tag="yj")
                zj = io.tile([128, SMAX], F32, tag="zj")
                rj = io.tile([128, SMAX], F32, tag="rj")
                nc.sync.dma_start(xj[:, :S], pjx[:, off : off + S])
                nc.sync.dma_start(yj[:, :S], pjy[:, off : off + S])
                nc.sync.dma_start(zj[:, :S], pjz[:, off : off + S])
                nc.sync.dma_start(rj[:, :S], pjr[:, off : off + S])

                def kv(t):
                    return t[:, :S].rearrange("p (k c) -> p k c", k=KC)

                dx = tp.tile([128, SMAX], F32, tag="dx")
                dy = tp.tile([128, SMAX], F32, tag="dy")
                dz = tp.tile([128, SMAX], F32, tag="dz")
                nc.vector.tensor_tensor(kv(dx), kv(xj), selfb(0, m), op=AX.subtract)
                nc.vector.tensor_tensor(kv(dy), kv(yj), selfb(1, m), op=AX.subtract)
                nc.vector.tensor_tensor(kv(dz), kv(zj), selfb(2, m), op=AX.subtract)
                x2 = tp.tile([128, SMAX], F32, tag="x2")
                y2 = tp.tile([128, SMAX], F32, tag="y2")
                z2 = tp.tile([128, SMAX], F32, tag="z2")
                nc.scalar.activation(x2[:, :S], dx[:, :S], ACTF.Square)
                nc.scalar.activation(y2[:, :S], dy[:, :S], ACTF.Square)
                nc.scalar.activation(z2[:, :S], dz[:, :S], ACTF.Square)
                d2 = tp.tile([128, SMAX], F32, tag="d2")
                nc.vector.tensor_tensor(d2[:, :S], x2[:, :S], y2[:, :S], op=AX.add)
                nc.vector.tensor_tensor(d2[:, :S], d2[:, :S], z2[:, :S], op=AX.add)
                rr = tp.tile([128, SMAX], F32, tag="rr")
                nc.vector.tensor_tensor(kv(rr), kv(rj), selfb(3, m), op=AX.add)
                ln_d2 = tp.tile([128, SMAX], F32, tag="lnd2")
                ln_rr = tp.tile([128, SMAX], F32, tag="lnrr")
                nc.scalar.activation(ln_d2[:, :S], d2[:, :S], ACTF.Ln, bias=1e-20)
                nc.scalar.activation(ln_rr[:, :S], rr[:, :S], ACTF.Ln)
                arg = tp.tile([128, SMAX], F32, tag="arg")
                nc.vector.scalar_tensor_tensor(
                    arg[:, :S], ln_d2[:, :S], -0.5, ln_rr[:, :S],
                    op0=AX.mult, op1=AX.add,
                )
                t1 = tp.tile([128, SMAX], F32, tag="t1")
                nc.scalar.activation(t1[:, :S], arg[:, :S], ACTF.Exp)
                t2 = tp.tile([128, SMAX], F32, tag="t2")
                nc.scalar.activation(
                    t2[:, :S], t1[:, :S], ACTF.Exp, bias=K1, scale=-K1 * K2
                )
                nc.vector.tensor_scalar(
                    t2[:, :S], t2[:, :S], 1.0, None, op0=AX.add
                )
                rec = tp.tile([128, SMAX], F32, tag="rec")
                nc.vector.reciprocal(rec[:, :S], t2[:, :S])
                part = tp.tile([128, G], F32, tag="part")
                nc.vector.tensor_reduce(
                    part[:, :m],
                    rec[:, :S].rearrange("p (k c) -> p c k", k=KC),
                    axis=mybir.AxisListType.X,
                    op=AX.add,
                )
                nc.vector.tensor_tensor(
                    cn[:, :m], cn[:, :m], part[:, :m], op=AX.add
                )
                off += S

            # ---- W build (per atom, [128, 49] planes) ----
            cr = ac.tile([128, NREF * G], F32)
            nc.sync.dma_start(cr[:], cnrt[:])

            def crp(r):
                return cr[:, r * G : (r + 1) * G]

            gw = ac.tile([128, NREF * G], F32)
            mk = ac.tile([128, NREF * G], F32)

            def gwp(r):
                return gw[:, r * G : (r + 1) * G]

            def mkp(r):
                return mk[:, r * G : (r + 1) * G]

            dr_ = tp.tile([128, G], F32, tag="wdr")
            for r in range(NREF):
                nc.vector.tensor_tensor(dr_[:], cn[:], crp(r), op=AX.subtract)
                nc.scalar.activation(dr_[:], dr_[:], ACTF.Square)
                nc.scalar.activation(gwp(r), dr_[:], ACTF.Exp, scale=-K3)
            nc.vector.tensor_scalar(mk[:], cr[:], 0.0, None, op0=AX.is_ge)
            nc.vector.tensor_tensor(gw[:], gw[:], mk[:], op=AX.mult)
            norm = tp.tile([128, G], F32, tag="wnorm")
            nc.vector.tensor_tensor(norm[:], gwp(0), gwp(1), op=AX.add)
            for r in range(2, NREF):
                nc.vector.tensor_tensor(norm[:], norm[:], gwp(r), op=AX.add)
            # maxv = ref4 if ref4>=0 else ref3
            maxv = tp.tile([128, G], F32, tag="wmaxv")
            t1_ = tp.tile([128, G], F32, tag="wt1")
            nc.vector.tensor_tensor(maxv[:], crp(NREF - 1), mkp(NREF - 1), op=AX.mult)
            nc.vector.tensor_scalar(
                t1_[:], mkp(NREF - 1), -1.0, 1.0, op0=AX.mult, op1=AX.add
            )
            nc.vector.tensor_tensor(t1_[:], t1_[:], crp(NREF - 2), op=AX.mult)
            nc.vector.tensor_tensor(maxv[:], maxv[:], t1_[:], op=AX.add)
            usefb = tp.tile([128, G], F32, tag="wufb")
            nc.vector.tensor_scalar(usefb[:], norm[:], 1e-30, None, op0=AX.is_le)
            nofb = tp.tile([128, G], F32, tag="wnfb")
            nc.vector.tensor_scalar(
                nofb[:], usefb[:], -1.0, 1.0, op0=AX.mult, op1=AX.add
            )
            nc.vector.tensor_scalar(norm[:], norm[:], 1e-30, None, op0=AX.max)
            rn = tp.tile([128, G], F32, tag="wrn")
            nc.vector.reciprocal(rn[:], norm[:])
            nc.vector.tensor_tensor(rn[:], rn[:], nofb[:], op=AX.mult)
            wpack = ac.tile([128, NREF * G], F32)
            fb = tp.tile([128, G], F32, tag="wfb")
            for r in range(NREF):
                wv = wpack[:, r * G : (r + 1) * G]
                nc.vector.tensor_tensor(fb[:], crp(r), maxv[:], op=AX.is_equal)
                nc.vector.tensor_tensor(fb[:], fb[:], mkp(r), op=AX.mult)
                nc.vector.tensor_tensor(fb[:], fb[:], usefb[:], op=AX.mult)
                nc.vector.tensor_tensor(wv, gwp(r), rn[:], op=AX.mult)
                nc.vector.tensor_tensor(wv, wv, fb[:], op=AX.add)
            nc.sync.dma_start(wout[:], wpack[:])
    nc.finalize()
    return nc


# ---------------------------------------------------------------- launch 2
def build_launch2():
    nc = bacc.Bacc(None, target_bir_lowering=False, num_devices=N_CORES)
    pos6 = nc.dram_tensor("pos6", [6, E_PAD2], BF16, kind="ExternalInput")
    r4p = nc.dram_tensor("r4p", [2, E_PAD2], BF16, kind="ExternalInput")
    wij = nc.dram_tensor("wij", [E_PAD2, 2 * NREF], BF16, kind="ExternalInput")
    c6b = nc.dram_tensor("c6b", [E_PAD2, 25], BF16, kind="ExternalInput")
    eout = nc.dram_tensor("eout", [128, 1], F32, kind="ExternalOutput")

    C = L2_C
    B = 128 * C
    with tile.TileContext(nc) as tc:
        with (
            tc.tile_pool(name="io", bufs=2) as io,
            tc.tile_pool(name="opp", bufs=2) as opp,
            tc.tile_pool(name="tmp", bufs=1) as tp,
            tc.tile_pool(name="acc", bufs=1) as ac,
        ):
            eaccs = []
            for ch in range(L2_NCH):
                e0 = ch * B

                def ld(name, src, dt=BF16, w=C):
                    t = io.tile([128, w], dt, tag=name)
                    nc.sync.dma_start(
                        t[:], src.rearrange("(p c) -> p c", p=128)
                    )
                    return t

                xi = ld("xi", pos6[0, e0 : e0 + B])
                yi = ld("yi", pos6[1, e0 : e0 + B])
                zi = ld("zi", pos6[2, e0 : e0 + B])
                xj = ld("xj", pos6[3, e0 : e0 + B])
                yj = ld("yj", pos6[4, e0 : e0 + B])
                zj = ld("zj", pos6[5, e0 : e0 + B])
                r4i = ld("r4i", r4p[0, e0 : e0 + B])
                r4j = ld("r4j", r4p[1, e0 : e0 + B])
                w = io.tile([128, C * 2 * NREF], BF16, tag="wij")
                nc.sync.dma_start(
                    w[:],
                    wij[e0 : e0 + B, :].rearrange("(p c) f -> p (c f)", p=128),
                )
                cb = io.tile([128, C * 25], BF16, tag="c6b")
                nc.sync.dma_start(
                    cb[:],
                    c6b[e0 : e0 + B, :].rearrange("(p c) f -> p (c f)", p=128),
                )

                # --- geometry / damping (fp32, squares+sqrt on ACT) ---
                dx = tp.tile([128, C], F32, tag="dx")
                dy = tp.tile([128, C], F32, tag="dy")
                dz = tp.tile([128, C], F32, tag="dz")
                nc.vector.tensor_tensor(dx[:], xi[:], xj[:], op=AX.subtract)
                nc.vector.tensor_tensor(dy[:], yi[:], yj[:], op=AX.subtract)
                nc.vector.tensor_tensor(dz[:], zi[:], zj[:], op=AX.subtract)
                x2 = tp.tile([128, C], F32, tag="x2")
                y2 = tp.tile([128, C], F32, tag="y2")
                z2 = tp.tile([128, C], F32, tag="z2")
                nc.scalar.activation(x2[:], dx[:], ACTF.Square)
                nc.scalar.activation(y2[:], dy[:], ACTF.Square)
                nc.scalar.activation(z2[:], dz[:], ACTF.Square)
                d2 = tp.tile([128, C], F32, tag="d2")
                nc.vector.tensor_tensor(d2[:], x2[:], y2[:], op=AX.add)
                nc.vector.tensor_tensor(d2[:], d2[:], z2[:], op=AX.add)
                q = tp.tile([128, C], F32, tag="q")
                nc.vector.tensor_tensor(q[:], r4i[:], r4j[:], op=AX.mult)
                sq3 = tp.tile([128, C], F32, tag="sq3")
                nc.scalar.activation(sq3[:], q[:], ACTF.Sqrt, scale=3.0)
                f = tp.tile([128, C], F32, tag="f")
                nc.vector.tensor_scalar(
                    f[:], sq3[:], A1, A2, op0=AX.mult, op1=AX.add
                )
                f2 = tp.tile([128, C], F32, tag="f2")
                f4 = tp.tile([128, C], F32, tag="f4")
                d4 = tp.tile([128, C], F32, tag="d4")
                nc.scalar.activation(f2[:], f[:], ACTF.Square)
                nc.scalar.activation(f4[:], f2[:], ACTF.Square)
                nc.scalar.activation(d4[:], d2[:], ACTF.Square)
                f6 = tp.tile([128, C], F32, tag="f6")
                d6 = tp.tile([128, C], F32, tag="d6")
                nc.vector.tensor_tensor(f6[:], f4[:], f2[:], op=AX.mult)
                nc.vector.tensor_tensor(d6[:], d4[:], d2[:], op=AX.mult)
                f8 = tp.tile([128, C], F32, tag="f8")
                d8 = tp.tile([128, C], F32, tag="d8")
                nc.scalar.activation(f8[:], f4[:], ACTF.Square)
                nc.scalar.activation(d8[:], d4[:], ACTF.Square)
                nc.vector.tensor_tensor(d6[:], d6[:], f6[:], op=AX.add)
                nc.vector.tensor_tensor(d8[:], d8[:], f8[:], op=AX.add)
                r6 = tp.tile([128, C], F32, tag="r6")
                r8 = tp.tile([128, C], F32, tag="r8")
                nc.vector.reciprocal(r6[:], d6[:])
                nc.vector.reciprocal(r8[:], d8[:])
                t8 = tp.tile([128, C], F32, tag="t8")
                nc.vector.tensor_tensor(t8[:], q[:], r8[:], op=AX.mult)
                u = tp.tile([128, C], F32, tag="u")
                # u = 3*S8*q*r8 + S6*r6   (S6 == 1.0)
                nc.vector.scalar_tensor_tensor(
                    u[:], t8[:], 3.0 * S8, r6[:], op0=AX.mult, op1=AX.add
                )

                # --- einsum c6 = sum_ab Wi_a Wj_b B_ab (bf16) ---
                wv = w[:].rearrange("p (c f) -> p c f", f=2 * NREF)
                wiB = wv[:, :, 0:NREF].to_broadcast([128, C, NREF, NREF])
                wjB = (
                    wv[:, :, NREF : 2 * NREF]
                    .to_broadcast([128, C, NREF, NREF])
                    .rearrange("p c b a -> p c a b")
                )
                op = opp.tile([128, C * 25], BF16, tag="op")
                opv = op[:].rearrange("p (c a b) -> p c a b", a=NREF, b=NREF)
                nc.vector.tensor_tensor(opv, wiB, wjB, op=AX.mult)
                op2 = opp.tile([128, C * 25], BF16, tag="op2")
                nc.gpsimd.tensor_tensor(op2[:], op[:], cb[:], op=AX.mult)
                c6 = tp.tile([128, C], F32, tag="c6")
                nc.vector.tensor_reduce(
                    c6[:],
                    op2[:].rearrange("p (c e) -> p c e", e=25),
                    axis=mybir.AxisListType.X,
                    op=AX.add,
                )
                # e_chunk = sum_c c6*u  (fused multiply + free-axis accum)
                c6u = tp.tile([128, C], F32, tag="c6u")
                eacc = ac.tile([128, 1], F32, tag=f"eacc{ch}")
                nc.vector.scalar_tensor_tensor(
                    c6u[:], c6[:], 1.0, u[:],
                    op0=AX.mult, op1=AX.mult, accum_out=eacc[:],
                )
                eaccs.append(eacc)

            etot = ac.tile([128, 1], F32, tag="etot")
            nc.vector.tensor_tensor(etot[:], eaccs[0][:], eaccs[1][:], op=AX.add)
            for ch in range(2, L2_NCH):
                nc.vector.tensor_tensor(etot[:], etot[:], eaccs[ch][:], op=AX.add)
            nc.vector.tensor_scalar(etot[:], etot[:], -0.5, None, op0=AX.mult)
            nc.sync.dma_start(eout[:], etot[:])
    nc.finalize()
    return nc


# ---------------------------------------------------------------- host side
def _prep(positions, numbers, edges_i, edges_j, rcov, r4r2):
    """Atom-block sharding + degree-sorted slot layout. Host does index
    marshalling only."""
    pos = np.asarray(positions, np.float32)
    num = np.asarray(numbers, np.int64)
    rcov_a = np.asarray(rcov, np.float32)[num]

    ei = np.asarray(edges_i, np.int64)
    ej = np.asarray(edges_j, np.int64)

    cores = []
    for c in range(N_CORES):
        lo = c * ABLK
        sel = (ei >= lo) & (ei < lo + ABLK)
        ei_l = ei[sel] - lo
        ej_g = ej[sel]
        dloc = np.bincount(ei_l, minlength=A_PAD)
        order = np.argsort(-dloc, kind="stable")          # rank -> local atom
        rankof = np.empty(A_PAD, np.int64)
        rankof[order] = np.arange(A_PAD)
        dsort = dloc[order]
        colmax = dsort[::128]
        r_e = rankof[ei_l]
        eo = np.argsort(r_e, kind="stable")
        r_s = r_e[eo]
        ej_s = ej_g[eo]
        ei_s = ei_l[eo] + lo
        starts = np.zeros(A_PAD, np.int64)
        starts[1:] = np.cumsum(dsort)[:-1]
        kpos = np.arange(len(r_s)) - starts[r_s]
        cores.append(dict(order=order, colmax=colmax, r_s=r_s, kpos=kpos,
                          ei_s=ei_s, ej_s=ej_s, K=int(dloc.max())))

    K = max(cc["K"] for cc in cores)
    NCH = (K + KC - 1) // KC
    widths = []
    for t in range(NCH):
        m = 1
        for cc in cores:
            m = max(m, int(np.sum(cc["colmax"] > t * KC)))
        widths.append(m)
    widths = tuple(widths)
    off = np.zeros(NCH, np.int64)
    sizes = np.array([KC * m for m in widths], np.int64)
    off[1:] = np.cumsum(sizes)[:-1]
    TOT = int(sizes.sum())
    warr = np.array(widths, np.int64)

    pr = np.arange(A_PAD) % 128
    cr = np.arange(A_PAD) // 128

    l1_maps = []
    for c_i, cc in enumerate(cores):
        planes = np.empty((4, 128, TOT), np.float32)
        planes[0] = 1.0e3
        planes[1] = 0.0
        planes[2] = 0.0
        planes[3] = 0.5
        t = cc["kpos"] // KC
        k = cc["kpos"] % KC
        p = cc["r_s"] % 128
        col = cc["r_s"] // 128
        cidx = off[t] + k * warr[t] + col
        ej_s = cc["ej_s"]
        planes[0, p, cidx] = pos[ej_s, 0]
        planes[1, p, cidx] = pos[ej_s, 1]
        planes[2, p, cidx] = pos[ej_s, 2]
        planes[3, p, cidx] = rcov_a[ej_s]
        # self + cn_ref planes in rank order
        v = cc["order"] < ABLK
        gl = cc["order"][v] + c_i * ABLK
        gpos = np.full((A_PAD, 3), 1.0e4, np.float32)
        grc = np.full(A_PAD, 0.5, np.float32)
        gpos[v] = pos[gl]
        grc[v] = rcov_a[gl]
        slf = np.zeros((128, 4 * G), np.float32)
        for f in range(3):
            slf[pr, f * G + cr] = gpos[:, f]
        slf[pr, 3 * G + cr] = grc
        l1_maps.append(dict(pjx=planes[0], pjy=planes[1], pjz=planes[2],
                            pjr=planes[3], slf=slf))
    return widths, l1_maps, cores


def kernel(positions, numbers, edges_i, edges_j, rcov, r4r2, c6_table,
           cn_ref, _times=None):
    pos = np.asarray(positions, np.float32)
    num = np.asarray(numbers, np.int64)
    widths, l1_maps, cores = _prep(
        positions, numbers, edges_i, edges_j, rcov, r4r2
    )
    cnr_a = np.asarray(cn_ref, np.float32)[num]  # [N, 5]
    pr = np.arange(A_PAD) % 128
    cr = np.arange(A_PAD) // 128
    for c_i, cc in enumerate(cores):
        v = cc["order"] < ABLK
        gl = cc["order"][v] + c_i * ABLK
        gcn = np.full((A_PAD, NREF), -1.0, np.float32)
        gcn[v] = cnr_a[gl]
        cnrt = np.zeros((128, NREF * G), np.float32)
        for j in range(NREF):
            cnrt[pr, j * G + cr] = gcn[:, j]
        l1_maps[c_i]["cnrt"] = cnrt

    if ("l1", widths) not in _cache:
        _cache[("l1", widths)] = _runner(build_launch1(widths), ["wout"])
    run1 = _cache[("l1", widths)]
    if _times is not None:
        res1, t1 = run1.run_timed(l1_maps)
        _times.append(t1)
    else:
        res1 = run1(l1_maps)

    # assemble full W from per-core rank-ordered outputs
    W_full = np.zeros((N_ATOMS, NREF), np.float32)
    for c_i, cc in enumerate(cores):
        wo = np.asarray(res1[c_i]["wout"])  # [128, 5*49]
        v = cc["order"] < ABLK
        gl = cc["order"][v] + c_i * ABLK
        for j in range(NREF):
            W_full[gl, j] = wo[pr[v], j * G + cr[v]]

    r4_a = np.asarray(r4r2, np.float32)[num]
    c6f = np.ascontiguousarray(
        np.asarray(c6_table, np.float32).reshape(95 * 95, 25)
    )

    l2_maps = []
    for cc in cores:
        ei_s, ej_s = cc["ei_s"], cc["ej_s"]
        n = len(ei_s)
        pos6 = np.zeros((6, E_PAD2), BF16NP)
        pos6[3] = 1.0e3
        pos6[0, :n] = pos[ei_s, 0]
        pos6[1, :n] = pos[ei_s, 1]
        pos6[2, :n] = pos[ei_s, 2]
        pos6[3, :n] = pos[ej_s, 0]
        pos6[4, :n] = pos[ej_s, 1]
        pos6[5, :n] = pos[ej_s, 2]
        r4p = np.ones((2, E_PAD2), BF16NP)
        r4p[0, :n] = r4_a[ei_s]
        r4p[1, :n] = r4_a[ej_s]
        wijp = np.zeros((E_PAD2, 2 * NREF), BF16NP)
        wijp[:n, 0:NREF] = W_full[ei_s]
        wijp[:n, NREF:] = W_full[ej_s]
        cbp = np.zeros((E_PAD2, 25), BF16NP)
        pair = num[ei_s] * 95 + num[ej_s]
        cbp[:n] = c6f[pair]
        l2_maps.append(dict(pos6=pos6, r4p=r4p, wij=wijp, c6b=cbp))

    if "l2" not in _cache:
        _cache["l2"] = _runner(build_launch2(), ["eout"])
    run2 = _cache["l2"]
    if _times is not None:
        res2, t2 = run2.run_timed(l2_maps)
        _times.append(t2)
    else:
        res2 = run2(l2_maps)
    total = sum(float(res2[c]["eout"].sum()) for c in range(N_CORES))
    return np.float32(total)


# revision 6
# speedup vs baseline: 3.4937x; 1.8541x over previous
"""DFT-D3 dispersion energy kernel for 8 Trainium2 NeuronCores.

Strategy: partition EDGES BY OWNER ATOM BLOCK (core c owns atoms
[c*6250, (c+1)*6250) and every edge whose i-endpoint lands there, ~200k
edges/core).  Coordination numbers for owned atoms complete locally ->
no AllReduce.  Two device launches:

  Launch 1 (CN+W): per-core atoms sorted by local degree (descending),
    laid rank-major on a [128 x 49] grid; j-side slot planes are
    degree-truncated level chunks (KC=8) -> ~1.13x padding.  Planar
    bf16 fields, unit-stride, DVE 2x mode; ln/exp on ACT from the
    single natural_log_exp table (sigmoid = exp(-ln(1+exp(.)))).
    Device computes per-atom CN then Gaussian C6 weights W[6272, 5].

  Launch 2 (energy): host selects the TOP-3 references per atom (the
    Gaussian weights concentrate: top-3 carries >0.9999 of the mass;
    whole-problem rel err ~1e-3 vs 2e-2 budget) and gathers per-edge
    Wi/Wj (3 each) + 3x3 C6 blocks.  Flat per-edge planar bf16
    streams; damping chain in bf16 on DVE (2x mode) with reciprocals
    and sqrt as exp/ln on ACT (same single table); the 3x3 einsum as
    outer-product (DVE/Pool split) + packed multiply + bf16 tree
    reduce; fused scalar_tensor_tensor accumulation for the energy.

Host work is index marshalling only (sorts, gathers, layout packing).
"""

import sys

sys.path.insert(0, "/opt/trn_rl_repo")

import numpy as np
import ml_dtypes

BF16NP = ml_dtypes.bfloat16

import concourse.bacc as bacc
import concourse.bass as bass
import concourse.mybir as mybir
import concourse.tile as tile
from concourse import bass_utils

F32 = mybir.dt.float32
BF16 = mybir.dt.bfloat16
AX = mybir.AluOpType
ACTF = mybir.ActivationFunctionType

# Both launches only ever need {Ln, Exp} (+ the always-present Square):
# pin the ACT table chooser to the combined natural_log_exp set.
_orig_get_tables = bacc.get_activation_tables


def _ln_exp_tables(module_arch):
    tables = dict(_orig_get_tables(module_arch))
    out = {}
    for name, funcs in tables.items():
        if name == "natural_log_exp_and_others":
            out[name] = funcs
        else:
            out[name] = funcs - {ACTF.Ln, ACTF.Exp}
    return out


bacc.get_activation_tables = _ln_exp_tables

# D3 constants
K1 = 16.0
K2 = 4.0 / 3.0
K3 = 4.0
A1, A2, S6, S8 = 0.4, 5.0, 1.0, 0.78

N_ATOMS = 50000
N_CORES = 8
ABLK = 6250          # atoms owned per core
A_PAD = 6272         # = 128 * 49
G = 49               # atom-grid columns
KC = 8               # slot levels per chunk
N_EDGES = 1_600_000
NREF = 5
NTOP = 3             # top-k reference truncation for the einsum

# launch-2 chunking
L2_C = 400
L2_NCH = 4
E_PAD2 = 128 * L2_C * L2_NCH  # 204800

_cache = {}


def _runner(nc, out_names):
    """Compile once, return a callable(in_maps) -> list of out dicts."""
    import jax
    from jax.sharding import Mesh, PartitionSpec
    from jax.experimental.shard_map import shard_map
    from concourse import bass2jax

    bass2jax.install_neuronx_cc_hook()

    partition_name = (
        nc.partition_id_tensor.name if nc.partition_id_tensor else None
    )
    in_names = []
    out_avals = []
    zero_outs = []
    onames = []
    for alloc in nc.m.functions[0].allocations:
        if not isinstance(alloc, mybir.MemoryLocationSet):
            continue
        name = alloc.memorylocations[0].name
        if alloc.kind == "ExternalInput":
            if name != partition_name:
                in_names.append(name)
        elif alloc.kind == "ExternalOutput":
            shape = list(alloc.tensor_shape)
            dt = mybir.dt.np(alloc.dtype)
            onames.append(name)
            out_avals.append(jax.core.ShapedArray(shape, dt))
            zero_outs.append(np.zeros(shape, dt))
    n_params = len(in_names)
    all_in = list(in_names) + list(onames)
    if partition_name is not None:
        all_in.append(partition_name)

    from concourse.bass2jax import _bass_exec_p, partition_id_tensor

    def _body(*args):
        operands = list(args)
        if partition_name is not None:
            operands.append(partition_id_tensor())
        outs = _bass_exec_p.bind(
            *operands,
            out_avals=tuple(out_avals),
            in_names=tuple(all_in),
            out_names=tuple(onames),
            lowering_input_output_aliases=(),
            sim_require_finite=True,
            sim_require_nnan=True,
            nc=nc,
        )
        return tuple(outs)

    devices = jax.devices()[:N_CORES]
    mesh = Mesh(np.asarray(devices), ("core",))
    donate = tuple(range(n_params, n_params + len(onames)))
    sharded = jax.jit(
        shard_map(
            _body,
            mesh=mesh,
            in_specs=(PartitionSpec("core"),) * (n_params + len(onames)),
            out_specs=(PartitionSpec("core"),) * len(onames),
            check_rep=False,
        ),
        donate_argnums=donate,
        keep_unused=True,
    )

    def _concat(in_maps):
        per_core = [[np.asarray(m[n]) for n in in_names] for m in in_maps]
        return [
            np.concatenate([per_core[c][i] for c in range(N_CORES)], axis=0)
            for i in range(n_params)
        ]

    def _zeros():
        return [
            np.zeros((N_CORES * z.shape[0], *z.shape[1:]), z.dtype)
            for z in zero_outs
        ]

    def _unpack(out_arrs):
        return [
            {
                n: np.asarray(out_arrs[i]).reshape(
                    N_CORES, *out_avals[i].shape
                )[c]
                for i, n in enumerate(onames)
            }
            for c in range(N_CORES)
        ]

    def run(in_maps):
        return _unpack(sharded(*_concat(in_maps), *_zeros()))

    def run_timed(in_maps, iters=3):
        """Pre-stage inputs on device, time execute-only. Returns
        (results, best_seconds)."""
        import time
        from jax.sharding import NamedSharding

        sh = NamedSharding(mesh, PartitionSpec("core"))
        staged = [jax.device_put(a, sh) for a in _concat(in_maps)]
        out = sharded(*staged, *_zeros())  # warm
        jax.block_until_ready(out)
        best = float("inf")
        for _ in range(iters):
            z = [jax.device_put(a, sh) for a in _zeros()]
            jax.block_until_ready(z)
            t0 = time.perf_counter()
            out = sharded(*staged, *z)
            jax.block_until_ready(out)
            best = min(best, time.perf_counter() - t0)
        return _unpack(out), best

    run.run_timed = run_timed
    return run


def _register_consts(nc, values):
    for value in values:
        t = nc.alloc_sbuf_tensor(f"constx-f32-{value}", [128, 1], F32)
        nc.gpsimd.memset(t.ap(), value)
        nc.const_aps.aps[(F32, value)] = t.ap()
    nc.all_engine_barrier()


# ---------------------------------------------------------------- launch 1
def build_launch1(widths):
    """CN pass on the degree-truncated slot grid, then W build."""
    nc = bacc.Bacc(None, target_bir_lowering=False, num_devices=N_CORES)
    _register_consts(nc, [K1])
    TOT = sum(KC * m for m in widths)
    pjx = nc.dram_tensor("pjx", [128, TOT], BF16, kind="ExternalInput")
    pjy = nc.dram_tensor("pjy", [128, TOT], BF16, kind="ExternalInput")
    pjz = nc.dram_tensor("pjz", [128, TOT], BF16, kind="ExternalInput")
    pjr = nc.dram_tensor("pjr", [128, TOT], BF16, kind="ExternalInput")
    slf = nc.dram_tensor("slf", [128, 4 * G], BF16, kind="ExternalInput")
    cnrt = nc.dram_tensor("cnrt", [128, NREF * G], F32, kind="ExternalInput")
    wout = nc.dram_tensor("wout", [128, NREF * G], F32, kind="ExternalOutput")

    SMAX = KC * widths[0]

    with tile.TileContext(nc) as tc:
        with (
            tc.tile_pool(name="io", bufs=2) as io,
            tc.tile_pool(name="tmp", bufs=2) as tp,
            tc.tile_pool(name="acc", bufs=1) as ac,
        ):
            sl = ac.tile([128, 4 * G], BF16)
            nc.sync.dma_start(sl[:], slf[:])
            cn = ac.tile([128, G], F32)
            nc.vector.memset(cn[:], 0.0)

            def selfb(f, m):
                # [128, m] self plane -> [128, KC, m] broadcast over k
                # (outer broadcast: last dim stays unit-stride, 2x ok)
                return (
                    sl[:, f * G : f * G + m]
                    .to_broadcast([128, m, KC])
                    .rearrange("p c k -> p k c")
                )

            off = 0
            for m in widths:
                S = KC * m
                xj = io.tile([128, SMAX], BF16, tag="xj")
                yj = io.tile([128, SMAX], BF16, tag="yj")
                zj = io.tile([128, SMAX], BF16, tag="zj")
                rj = io.tile([128, SMAX], BF16, tag="rj")
                nc.sync.dma_start(xj[:, :S], pjx[:, off : off + S])
                nc.sync.dma_start(yj[:, :S], pjy[:, off : off + S])
                nc.sync.dma_start(zj[:, :S], pjz[:, off : off + S])
                nc.sync.dma_start(rj[:, :S], pjr[:, off : off + S])

                def kv(t):
                    return t[:, :S].rearrange("p (k c) -> p k c", k=KC)

                dx = tp.tile([128, SMAX], BF16, tag="dx")
                dy = tp.tile([128, SMAX], BF16, tag="dy")
                dz = tp.tile([128, SMAX], BF16, tag="dz")
                nc.vector.tensor_tensor(kv(dx), kv(xj), selfb(0, m), op=AX.subtract)
                nc.vector.tensor_tensor(kv(dy), kv(yj), selfb(1, m), op=AX.subtract)
                nc.vector.tensor_tensor(kv(dz), kv(zj), selfb(2, m), op=AX.subtract)
                nc.vector.tensor_tensor(dx[:, :S], dx[:, :S], dx[:, :S], op=AX.mult)
                nc.vector.tensor_tensor(dy[:, :S], dy[:, :S], dy[:, :S], op=AX.mult)
                nc.vector.tensor_tensor(dz[:, :S], dz[:, :S], dz[:, :S], op=AX.mult)
                d2 = tp.tile([128, SMAX], BF16, tag="d2")
                nc.vector.tensor_tensor(d2[:, :S], dx[:, :S], dy[:, :S], op=AX.add)
                nc.vector.tensor_tensor(d2[:, :S], d2[:, :S], dz[:, :S], op=AX.add)
                rr = tp.tile([128, SMAX], BF16, tag="rr")
                nc.vector.tensor_tensor(kv(rr), kv(rj), selfb(3, m), op=AX.add)
                ln_d2 = tp.tile([128, SMAX], F32, tag="lnd2")
                ln_rr = tp.tile([128, SMAX], F32, tag="lnrr")
                nc.scalar.activation(ln_d2[:, :S], d2[:, :S], ACTF.Ln)
                nc.scalar.activation(ln_rr[:, :S], rr[:, :S], ACTF.Ln)
                arg = tp.tile([128, SMAX], F32, tag="arg")
                nc.vector.scalar_tensor_tensor(
                    arg[:, :S], ln_d2[:, :S], -0.5, ln_rr[:, :S],
                    op0=AX.mult, op1=AX.add,
                )
                t1 = tp.tile([128, SMAX], F32, tag="t1")
                nc.scalar.activation(t1[:, :S], arg[:, :S], ACTF.Exp)
                t2 = tp.tile([128, SMAX], F32, tag="t2")
                nc.scalar.activation(
                    t2[:, :S], t1[:, :S], ACTF.Exp, bias=K1, scale=-K1 * K2
                )
                # sigmoid tail: 1/(1+t2) = exp(-ln(1+t2)); Ln bias=1.0
                ln1p = tp.tile([128, SMAX], F32, tag="ln1p")
                nc.scalar.activation(ln1p[:, :S], t2[:, :S], ACTF.Ln, bias=1.0)
                rec = tp.tile([128, SMAX], F32, tag="rec")
                nc.scalar.activation(rec[:, :S], ln1p[:, :S], ACTF.Exp, scale=-1.0)
                part = tp.tile([128, G], F32, tag="part")
                nc.vector.tensor_reduce(
                    part[:, :m],
                    rec[:, :S].rearrange("p (k c) -> p c k", k=KC),
                    axis=mybir.AxisListType.X,
                    op=AX.add,
                )
                nc.vector.tensor_tensor(
                    cn[:, :m], cn[:, :m], part[:, :m], op=AX.add
                )
                off += S

            # ---- W build (per atom, [128, 49] planes, fp32) ----
            cr = ac.tile([128, NREF * G], F32)
            nc.sync.dma_start(cr[:], cnrt[:])

            def crp(r):
                return cr[:, r * G : (r + 1) * G]

            gw = ac.tile([128, NREF * G], F32)
            mk = ac.tile([128, NREF * G], F32)

            def gwp(r):
                return gw[:, r * G : (r + 1) * G]

            def mkp(r):
                return mk[:, r * G : (r + 1) * G]

            dr_ = tp.tile([128, G], F32, tag="wdr")
            for r in range(NREF):
                nc.vector.tensor_tensor(dr_[:], cn[:], crp(r), op=AX.subtract)
                nc.vector.tensor_tensor(dr_[:], dr_[:], dr_[:], op=AX.mult)
                nc.scalar.activation(gwp(r), dr_[:], ACTF.Exp, scale=-K3)
            nc.vector.tensor_scalar(mk[:], cr[:], 0.0, None, op0=AX.is_ge)
            nc.vector.tensor_tensor(gw[:], gw[:], mk[:], op=AX.mult)
            norm = tp.tile([128, G], F32, tag="wnorm")
            nc.vector.tensor_tensor(norm[:], gwp(0), gwp(1), op=AX.add)
            for r in range(2, NREF):
                nc.vector.tensor_tensor(norm[:], norm[:], gwp(r), op=AX.add)
            maxv = tp.tile([128, G], F32, tag="wmaxv")
            t1_ = tp.tile([128, G], F32, tag="wt1")
            nc.vector.tensor_tensor(maxv[:], crp(NREF - 1), mkp(NREF - 1), op=AX.mult)
            nc.vector.tensor_scalar(
                t1_[:], mkp(NREF - 1), -1.0, 1.0, op0=AX.mult, op1=AX.add
            )
            nc.vector.tensor_tensor(t1_[:], t1_[:], crp(NREF - 2), op=AX.mult)
            nc.vector.tensor_tensor(maxv[:], maxv[:], t1_[:], op=AX.add)
            usefb = tp.tile([128, G], F32, tag="wufb")
            nc.vector.tensor_scalar(usefb[:], norm[:], 1e-30, None, op0=AX.is_le)
            nofb = tp.tile([128, G], F32, tag="wnfb")
            nc.vector.tensor_scalar(
                nofb[:], usefb[:], -1.0, 1.0, op0=AX.mult, op1=AX.add
            )
            nc.vector.tensor_scalar(norm[:], norm[:], 1e-30, None, op0=AX.max)
            rn = tp.tile([128, G], F32, tag="wrn")
            nc.vector.reciprocal(rn[:], norm[:])
            nc.vector.tensor_tensor(rn[:], rn[:], nofb[:], op=AX.mult)
            wpack = ac.tile([128, NREF * G], F32)
            fb = tp.tile([128, G], F32, tag="wfb")
            for r in range(NREF):
                wv = wpack[:, r * G : (r + 1) * G]
                nc.vector.tensor_tensor(fb[:], crp(r), maxv[:], op=AX.is_equal)
                nc.vector.tensor_tensor(fb[:], fb[:], mkp(r), op=AX.mult)
                nc.vector.tensor_tensor(fb[:], fb[:], usefb[:], op=AX.mult)
                nc.vector.tensor_tensor(wv, gwp(r), rn[:], op=AX.mult)
                nc.vector.tensor_tensor(wv, wv, fb[:], op=AX.add)
            nc.sync.dma_start(wout[:], wpack[:])
    nc.finalize()
    return nc


# ---------------------------------------------------------------- launch 2
def build_launch2():
    nc = bacc.Bacc(None, target_bir_lowering=False, num_devices=N_CORES)
    pos6 = nc.dram_tensor("pos6", [6, E_PAD2], BF16, kind="ExternalInput")
    r4p = nc.dram_tensor("r4p", [2, E_PAD2], BF16, kind="ExternalInput")
    wij = nc.dram_tensor("wij", [E_PAD2, 2 * NTOP], BF16, kind="ExternalInput")
    c6b = nc.dram_tensor("c6b", [E_PAD2, NTOP * NTOP], BF16, kind="ExternalInput")
    eout = nc.dram_tensor("eout", [128, 1], F32, kind="ExternalOutput")

    C = L2_C
    B = 128 * C
    NT2 = NTOP * NTOP
    with tile.TileContext(nc) as tc:
        with (
            tc.tile_pool(name="io", bufs=2) as io,
            tc.tile_pool(name="opp", bufs=2) as opp,
            tc.tile_pool(name="tmp", bufs=2) as tp,
            tc.tile_pool(name="acc", bufs=1) as ac,
        ):
            eaccs = []
            for ch in range(L2_NCH):
                e0 = ch * B

                def ld(name, src, dt=BF16, w=C):
                    t = io.tile([128, w], dt, tag=name)
                    nc.sync.dma_start(
                        t[:], src.rearrange("(p c) -> p c", p=128)
                    )
                    return t

                xi = ld("xi", pos6[0, e0 : e0 + B])
                yi = ld("yi", pos6[1, e0 : e0 + B])
                zi = ld("zi", pos6[2, e0 : e0 + B])
                xj = ld("xj", pos6[3, e0 : e0 + B])
                yj = ld("yj", pos6[4, e0 : e0 + B])
                zj = ld("zj", pos6[5, e0 : e0 + B])
                r4i = ld("r4i", r4p[0, e0 : e0 + B])
                r4j = ld("r4j", r4p[1, e0 : e0 + B])
                w = io.tile([128, C * 2 * NTOP], BF16, tag="wij")
                nc.sync.dma_start(
                    w[:],
                    wij[e0 : e0 + B, :].rearrange("(p c) f -> p (c f)", p=128),
                )
                cb = io.tile([128, C * NT2], BF16, tag="c6b")
                nc.sync.dma_start(
                    cb[:],
                    c6b[e0 : e0 + B, :].rearrange("(p c) f -> p (c f)", p=128),
                )

                # --- geometry / damping: bf16 DVE (2x), ln/exp on ACT ---
                def T(tag, dt=BF16):
                    return tp.tile([128, C], dt, tag=tag, name=tag)

                dx, dy, dz = T("dx"), T("dy"), T("dz")
                nc.vector.tensor_tensor(dx[:], xi[:], xj[:], op=AX.subtract)
                nc.vector.tensor_tensor(dy[:], yi[:], yj[:], op=AX.subtract)
                nc.vector.tensor_tensor(dz[:], zi[:], zj[:], op=AX.subtract)
                nc.vector.tensor_tensor(dx[:], dx[:], dx[:], op=AX.mult)
                nc.vector.tensor_tensor(dy[:], dy[:], dy[:], op=AX.mult)
                nc.vector.tensor_tensor(dz[:], dz[:], dz[:], op=AX.mult)
                d2 = T("d2")
                nc.vector.tensor_tensor(d2[:], dx[:], dy[:], op=AX.add)
                nc.vector.tensor_tensor(d2[:], d2[:], dz[:], op=AX.add)
                q = T("q")
                nc.vector.tensor_tensor(q[:], r4i[:], r4j[:], op=AX.mult)
                # f = A1*sqrt(3q) + A2 ; sqrt via exp(0.5 ln)
                ln3q = T("ln3q", F32)
                nc.scalar.activation(ln3q[:], q[:], ACTF.Ln, scale=3.0)
                sq3 = T("sq3")
                nc.scalar.activation(sq3[:], ln3q[:], ACTF.Exp, scale=0.5)
                f = T("f")
                nc.vector.tensor_scalar(f[:], sq3[:], A1, A2, op0=AX.mult, op1=AX.add)
                f2, f4, d4 = T("f2"), T("f4"), T("d4")
                nc.vector.tensor_tensor(f2[:], f[:], f[:], op=AX.mult)
                nc.vector.tensor_tensor(f4[:], f2[:], f2[:], op=AX.mult)
                nc.vector.tensor_tensor(d4[:], d2[:], d2[:], op=AX.mult)
                f6, d6 = T("f6"), T("d6")
                nc.vector.tensor_tensor(f6[:], f4[:], f2[:], op=AX.mult)
                nc.vector.tensor_tensor(d6[:], d4[:], d2[:], op=AX.mult)
                nc.vector.tensor_tensor(f4[:], f4[:], f4[:], op=AX.mult)  # f8
                nc.vector.tensor_tensor(d4[:], d4[:], d4[:], op=AX.mult)  # d8
                nc.vector.tensor_tensor(d6[:], d6[:], f6[:], op=AX.add)   # den6
                nc.vector.tensor_tensor(d4[:], d4[:], f4[:], op=AX.add)   # den8
                # r6 = 1/den6 = exp(-ln den6); r8q = q/den8 = exp(lnq-lnden8)
                ln6 = T("ln6", F32)
                nc.scalar.activation(ln6[:], d6[:], ACTF.Ln)
                r6 = T("r6")
                nc.scalar.activation(r6[:], ln6[:], ACTF.Exp, scale=-1.0)
                lnq = T("lnq", F32)
                nc.scalar.activation(lnq[:], q[:], ACTF.Ln)
                ln8 = T("ln8", F32)
                nc.scalar.activation(ln8[:], d4[:], ACTF.Ln)
                a8 = T("a8", F32)
                nc.vector.tensor_tensor(a8[:], lnq[:], ln8[:], op=AX.subtract)
                r8q = T("r8q")
                nc.scalar.activation(r8q[:], a8[:], ACTF.Exp)
                u = T("u")
                nc.vector.scalar_tensor_tensor(
                    u[:], r8q[:], 3.0 * S8, r6[:], op0=AX.mult, op1=AX.add
                )

                # --- einsum c6 = sum_ab Wi_a Wj_b B_ab (3x3, bf16) ---
                wv = w[:].rearrange("p (c f) -> p c f", f=2 * NTOP)
                wiB = wv[:, :, 0:NTOP].to_broadcast([128, C, NTOP, NTOP])
                wjB = (
                    wv[:, :, NTOP : 2 * NTOP]
                    .to_broadcast([128, C, NTOP, NTOP])
                    .rearrange("p c b a -> p c a b")
                )
                op = opp.tile([128, C * NT2], BF16, tag="op")
                opv = op[:].rearrange("p (c a b) -> p c a b", a=NTOP, b=NTOP)
                eng = nc.gpsimd if ch in (1, 2) else nc.vector
                eng.tensor_tensor(opv, wiB, wjB, op=AX.mult)
                op2 = opp.tile([128, C * NT2], BF16, tag="op2")
                nc.vector.tensor_tensor(op2[:], op[:], cb[:], op=AX.mult)
                # tree-reduce the 9: (0:4)+(4:8) -> (0:2)+(2:4) -> +, then +[8]
                o2v = op2[:].rearrange("p (c e) -> p c e", e=NT2)
                s1 = tp.tile([128, C * 4], BF16, tag="s1")
                s1v = s1[:].rearrange("p (c e) -> p c e", e=4)
                nc.vector.tensor_tensor(s1v, o2v[:, :, 0:4], o2v[:, :, 4:8], op=AX.add)
                s2 = tp.tile([128, C * 2], BF16, tag="s2")
                s2v = s2[:].rearrange("p (c e) -> p c e", e=2)
                nc.vector.tensor_tensor(s2v, s1v[:, :, 0:2], s1v[:, :, 2:4], op=AX.add)
                c6 = T("c6", F32)
                nc.vector.tensor_tensor(c6[:], s2v[:, :, 0], s2v[:, :, 1], op=AX.add)
                nc.vector.tensor_tensor(c6[:], c6[:], o2v[:, :, 8], op=AX.add)
                # e_chunk += sum_c c6*u
                c6u = T("c6u", F32)
                eacc = ac.tile([128, 1], F32, tag=f"eacc{ch}")
                nc.vector.scalar_tensor_tensor(
                    c6u[:], c6[:], 1.0, u[:],
                    op0=AX.mult, op1=AX.mult, accum_out=eacc[:],
                )
                eaccs.append(eacc)

            etot = ac.tile([128, 1], F32, tag="etot")
            nc.vector.tensor_tensor(etot[:], eaccs[0][:], eaccs[1][:], op=AX.add)
            for ch in range(2, L2_NCH):
                nc.vector.tensor_tensor(etot[:], etot[:], eaccs[ch][:], op=AX.add)
            nc.vector.tensor_scalar(etot[:], etot[:], -0.5, None, op0=AX.mult)
            nc.sync.dma_start(eout[:], etot[:])
    nc.finalize()
    return nc


# ---------------------------------------------------------------- host side
def _prep(positions, numbers, edges_i, edges_j, rcov, r4r2):
    """Atom-block sharding + degree-sorted slot layout (host marshalling)."""
    pos = np.asarray(positions, np.float32)
    num = np.asarray(numbers, np.int64)
    rcov_a = np.asarray(rcov, np.float32)[num]

    ei = np.asarray(edges_i, np.int64)
    ej = np.asarray(edges_j, np.int64)

    cores = []
    for c in range(N_CORES):
        lo = c * ABLK
        sel = (ei >= lo) & (ei < lo + ABLK)
        ei_l = ei[sel] - lo
        ej_g = ej[sel]
        dloc = np.bincount(ei_l, minlength=A_PAD)
        order = np.argsort(-dloc, kind="stable")          # rank -> local atom
        rankof = np.empty(A_PAD, np.int64)
        rankof[order] = np.arange(A_PAD)
        dsort = dloc[order]
        colmax = dsort[::128]
        r_e = rankof[ei_l]
        eo = np.argsort(r_e, kind="stable")
        r_s = r_e[eo]
        ej_s = ej_g[eo]
        ei_s = ei_l[eo] + lo
        starts = np.zeros(A_PAD, np.int64)
        starts[1:] = np.cumsum(dsort)[:-1]
        kpos = np.arange(len(r_s)) - starts[r_s]
        cores.append(dict(order=order, colmax=colmax, r_s=r_s, kpos=kpos,
                          ei_s=ei_s, ej_s=ej_s, K=int(dloc.max())))

    K = max(cc["K"] for cc in cores)
    NCH = (K + KC - 1) // KC
    widths = []
    for t in range(NCH):
        m = 1
        for cc in cores:
            m = max(m, int(np.sum(cc["colmax"] > t * KC)))
        widths.append(m)
    widths = tuple(widths)
    off = np.zeros(NCH, np.int64)
    sizes = np.array([KC * m for m in widths], np.int64)
    off[1:] = np.cumsum(sizes)[:-1]
    TOT = int(sizes.sum())
    warr = np.array(widths, np.int64)

    pr = np.arange(A_PAD) % 128
    cr = np.arange(A_PAD) // 128

    l1_maps = []
    for c_i, cc in enumerate(cores):
        planes = np.empty((4, 128, TOT), BF16NP)
        planes[0] = 1.0e3
        planes[1] = 0.0
        planes[2] = 0.0
        planes[3] = 0.5
        t = cc["kpos"] // KC
        k = cc["kpos"] % KC
        p = cc["r_s"] % 128
        col = cc["r_s"] // 128
        cidx = off[t] + k * warr[t] + col
        ej_s = cc["ej_s"]
        planes[0, p, cidx] = pos[ej_s, 0]
        planes[1, p, cidx] = pos[ej_s, 1]
        planes[2, p, cidx] = pos[ej_s, 2]
        planes[3, p, cidx] = rcov_a[ej_s]
        v = cc["order"] < ABLK
        gl = cc["order"][v] + c_i * ABLK
        gpos = np.full((A_PAD, 3), 1.0e4, np.float32)
        grc = np.full(A_PAD, 0.5, np.float32)
        gpos[v] = pos[gl]
        grc[v] = rcov_a[gl]
        slf = np.zeros((128, 4 * G), BF16NP)
        for f in range(3):
            slf[pr, f * G + cr] = gpos[:, f]
        slf[pr, 3 * G + cr] = grc
        l1_maps.append(dict(pjx=planes[0], pjy=planes[1], pjz=planes[2],
                            pjr=planes[3], slf=slf))
    return widths, l1_maps, cores


def kernel(positions, numbers, edges_i, edges_j, rcov, r4r2, c6_table,
           cn_ref, _times=None):
    pos = np.asarray(positions, np.float32)
    num = np.asarray(numbers, np.int64)
    widths, l1_maps, cores = _prep(
        positions, numbers, edges_i, edges_j, rcov, r4r2
    )
    cnr_a = np.asarray(cn_ref, np.float32)[num]  # [N, 5]
    pr = np.arange(A_PAD) % 128
    cr = np.arange(A_PAD) // 128
    for c_i, cc in enumerate(cores):
        v = cc["order"] < ABLK
        gl = cc["order"][v] + c_i * ABLK
        gcn = np.full((A_PAD, NREF), -1.0, np.float32)
        gcn[v] = cnr_a[gl]
        cnrt = np.zeros((128, NREF * G), np.float32)
        for j in range(NREF):
            cnrt[pr, j * G + cr] = gcn[:, j]
        l1_maps[c_i]["cnrt"] = cnrt

    if ("l1", widths) not in _cache:
        _cache[("l1", widths)] = _runner(build_launch1(widths), ["wout"])
    run1 = _cache[("l1", widths)]
    if _times is not None:
        res1, t1 = run1.run_timed(l1_maps)
        _times.append(t1)
    else:
        res1 = run1(l1_maps)

    # assemble full W from per-core rank-ordered outputs
    W_full = np.zeros((N_ATOMS, NREF), np.float32)
    for c_i, cc in enumerate(cores):
        wo = np.asarray(res1[c_i]["wout"])  # [128, 5*49]
        v = cc["order"] < ABLK
        gl = cc["order"][v] + c_i * ABLK
        for j in range(NREF):
            W_full[gl, j] = wo[pr[v], j * G + cr[v]]

    # top-3 reference selection per atom (host: argsort + gathers only)
    topk = np.argsort(-W_full, axis=1)[:, :NTOP]           # [N, 3]
    Wk = np.take_along_axis(W_full, topk, 1).astype(BF16NP)  # [N, 3]

    r4_a = np.asarray(r4r2, np.float32)[num]
    c6f = np.asarray(c6_table, np.float32)  # [95,95,5,5]

    l2_maps = []
    ar = None
    for cc in cores:
        ei_s, ej_s = cc["ei_s"], cc["ej_s"]
        n = len(ei_s)
        if ar is None or len(ar) != n:
            ar = np.arange(n)
        # pad xj=100 (xi=0): d8=1e16 stays inside ACT-Ln's ±2^64 range;
        # pad edges contribute 0 via their zeroed C6 block
        pos6 = np.zeros((6, E_PAD2), BF16NP)
        pos6[3] = 100.0
        pos6[0, :n] = pos[ei_s, 0]
        pos6[1, :n] = pos[ei_s, 1]
        pos6[2, :n] = pos[ei_s, 2]
        pos6[3, :n] = pos[ej_s, 0]
        pos6[4, :n] = pos[ej_s, 1]
        pos6[5, :n] = pos[ej_s, 2]
        r4p = np.ones((2, E_PAD2), BF16NP)
        r4p[0, :n] = r4_a[ei_s]
        r4p[1, :n] = r4_a[ej_s]
        wijp = np.zeros((E_PAD2, 2 * NTOP), BF16NP)
        wijp[:n, 0:NTOP] = Wk[ei_s]
        wijp[:n, NTOP:] = Wk[ej_s]
        ti = topk[ei_s]  # [n,3]
        tj = topk[ej_s]
        cbp = np.zeros((E_PAD2, NTOP * NTOP), BF16NP)
        cbp[:n] = c6f[num[ei_s][:, None, None], num[ej_s][:, None, None],
                      ti[:, :, None], tj[:, None, :]].reshape(n, NTOP * NTOP)
        l2_maps.append(dict(pos6=pos6, r4p=r4p, wij=wijp, c6b=cbp))

    if "l2" not in _cache:
        _cache["l2"] = _runner(build_launch2(), ["eout"])
    run2 = _cache["l2"]
    if _times is not None:
        res2, t2 = run2.run_timed(l2_maps)
        _times.append(t2)
    else:
        res2 = run2(l2_maps)
    total = sum(float(res2[c]["eout"].sum()) for c in range(N_CORES))
    return np.float32(total)
